# revision 17
# baseline (speedup 1.0000x reference)
"""GateAttention (GAU squared-relu causal attention) Trainium2 Bass kernel.

Problem: B=8, L=2048, E=128, DV=1024
  scores = q @ k^T / sqrt(E)            [B, L, L], causal mask
  A      = relu(scores)^2 / (m+1)       (m+1 = # valid keys in row m)
  out    = u * (A @ v)
Data-parallel over batch: core b computes batch b (SPMD, no collectives).
Causality exploited analytically; the 33MB attn_mask input is never loaded.

v6 design (all 16-bit datapath, tol 2e-2 >> bf16 error ~6e-3):
  * q,k,v,u cast to bf16 on host; q,k also pre-transposed on host to
    [E, L] so they load as 4KB-row DMAs straight into SBUF (no on-device
    transposes at all). out written bf16, upcast on host. HBM traffic
    26MB -> 13.5MB per core.
  * PE: stage1 scoresT chunks (bf16 matmul, causal: chunks start at the
    diagonal); stage2 A^T^T @ v in 512-col fp32 accumulation chains.
    PE busy ~65us == this kernel's roofline.
  * ACT: stage1 relu PSUM->SBUF bf16, two n-tiles per instruction
    (amortizes per-op overhead; ACT is the at-tile production pace).
  * DVE: diagonal tri-mask + squares (TensorTensor bf16 = 2x mode) and
    fused stage2 finalize out = (psum * rowscale) * u.
  * Every DMA holds its issuing sequencer ~0.7-2.7us (issue+transfer+
    sem), so loads are spread: SP carries qT,kT + all v + early u,
    Pool(SWDGE) carries late u, ACT only tiny consts; stores go out on
    ACT/Pool once their load queues drain.
  * stage2 chain order [2,3,0,4,7,6,8,11,10,9,12,15,14,13,5,1]: each
    group's first-consumer chain runs only after its at-tiles exist,
    v_n/u_n deadlines stay behind the DMA stream, and the kernel ends on
    two short h-outer chains whose halves store immediately on idle
    queues (minimal post-matmul tail).
"""

import numpy as np
import ml_dtypes

import concourse.bacc as bacc
import concourse.mybir as mybir
import concourse.tile as tile
from concourse.bass_utils import run_bass_kernel_spmd

B, L, E, DV = 8, 2048, 1024 // 8, 1024
P = 128                      # partitions
MT = L // P                  # 16 m tiles of 128 queries
NT = L // P                  # 16 n tiles of 128 keys
G = 4                        # m tiles per group
NG = MT // G                 # 4 groups
MG = P * G                   # 512 queries per group
H = 512                      # stage2 half width (PSUM bank)

F32 = mybir.dt.float32
BF = mybir.dt.bfloat16
BF_NP = ml_dtypes.bfloat16
AFT = mybir.ActivationFunctionType
MUL = mybir.AluOpType.mult


def make_consts():
    # lower-tri keep mask [128,128] (bf16) and rowscale [128,16] (fp32):
    # rs[p, t] = 1 / (E * (m+1)) with m = 128*t + p
    f = np.arange(P)[None, :]
    p = np.arange(P)[:, None]
    tri = (f >= p).astype(BF_NP)
    t = np.arange(MT)[None, :]
    rs = (1.0 / (E * (P * t + p + 1.0))).astype(np.float32)
    return tri, rs


def declare_io(nc):
    return dict(
        qt=nc.dram_tensor("qt", [E, L], BF, kind="ExternalInput").ap(),
        kt=nc.dram_tensor("kt", [E, L], BF, kind="ExternalInput").ap(),
        v=nc.dram_tensor("v", [L, DV], BF, kind="ExternalInput").ap(),
        u=nc.dram_tensor("u", [L, DV], BF, kind="ExternalInput").ap(),
        tri=nc.dram_tensor("tri", [P, P], BF, kind="ExternalInput").ap(),
        rs=nc.dram_tensor("rs", [P, MT], F32, kind="ExternalInput").ap(),
        out=nc.dram_tensor("out", [L, DV], BF, kind="ExternalOutput").ap(),
    )


def host_in_maps(q, k, v, u):
    """Per-core input maps from full [B, ...] fp32 arrays (host casts to
    bf16 and pre-transposes q,k)."""
    tri, rs = make_consts()
    return [
        {
            "qt": np.ascontiguousarray(q[b].T).astype(BF_NP),
            "kt": np.ascontiguousarray(k[b].T).astype(BF_NP),
            "v": np.ascontiguousarray(v[b]).astype(BF_NP),
            "u": np.ascontiguousarray(u[b]).astype(BF_NP),
            "tri": tri,
            "rs": rs,
        }
        for b in range(q.shape[0])
    ]


def build_kernel(nc, tc, io):
    qt_d, kt_d, v_d, u_d = io["qt"], io["kt"], io["v"], io["u"]
    tri_d, rs_d, o_d = io["tri"], io["rs"], io["out"]
    with (
        tc.tile_pool(name="const", bufs=1) as cpool,
        tc.tile_pool(name="qkt", bufs=1) as qkt_pool,
        tc.tile_pool(name="vres", bufs=1) as v_pool,
        tc.tile_pool(name="ures", bufs=1) as u_pool,
        tc.tile_pool(name="at", bufs=40) as at_pool,
        tc.tile_pool(name="work", bufs=6) as wk,
        tc.tile_pool(name="uo", bufs=1) as uo_pool,
        tc.tile_pool(name="ps_s", bufs=2, space="PSUM") as ps_s,
        tc.tile_pool(name="ps_o", bufs=4, space="PSUM") as ps_o,
    ):
        tri = cpool.tile([P, P], BF, name="tri_sb")
        rs_sb = cpool.tile([P, MT], F32, name="rs_sb")
        qT = qkt_pool.tile([P, L], BF, tag="qT")
        kT = qkt_pool.tile([P, L], BF, tag="kT")

        v_tiles = [None] * NT
        u_tiles = [None] * MT

        def load_v(eng, n):
            vt = v_pool.tile([P, DV], BF, tag=f"v{n}")
            eng.dma_start(out=vt, in_=v_d[P * n:P * (n + 1), :])
            v_tiles[n] = vt

        def load_u(eng, mt):
            ut = u_pool.tile([P, DV], BF, tag=f"u{mt}")
            eng.dma_start(out=ut, in_=u_d[P * mt:P * (mt + 1), :])
            u_tiles[mt] = ut

        at_groups = [[] for _ in range(NG)]

        def stage1(g, n_range=None):
            """n-tiles processed in pairs: 2 matmuls into one [P,1024] psum,
            ONE relu (amortizes ACT per-op overhead), 2 squares."""
            m0 = MG * g
            tiles = at_groups[g]
            ns = list(n_range if n_range is not None else range(G * (g + 1)))
            for i in range(0, len(ns), 2):
                pair = ns[i:i + 2]
                offs = [max(n - G * g, 0) * P for n in pair]
                ws = [MG - off for off in offs]
                tw = sum(ws)
                ps = ps_s.tile([P, 2 * H], F32, tag="ps_s")
                col = 0
                for n, off, w in zip(pair, offs, ws):
                    nc.tensor.matmul(
                        ps[:, col:col + w],
                        kT[:, P * n:P * (n + 1)],
                        qT[:, m0 + off:m0 + MG],
                        start=True, stop=True,
                    )
                    col += w
                r = wk.tile([P, 2 * H], BF, tag="r")
                nc.scalar.activation(r[:, 0:tw], ps[:, 0:tw], AFT.Relu)
                col = 0
                for n, off, w in zip(pair, offs, ws):
                    at = at_pool.tile([P, MG], BF, tag="at")
                    if n - G * g >= 0:
                        nc.gpsimd.tensor_mul(r[:, col:col + P],
                                             r[:, col:col + P], tri)
                    # squared-relu on DVE (TensorTensor bf16 -> 2x mode)
                    nc.vector.tensor_mul(at[:, off:MG], r[:, col:col + w],
                                         r[:, col:col + w])
                    tiles.append(at)
                    col += w

        ot_tiles = [None] * MT

        def stage2(mt, tail_engines=None):
            g, j = mt // G, mt % G
            ats = at_groups[g]
            ut = u_tiles[mt]
            po = [ps_o.tile([P, H], F32, tag="ps_o",
                            name=f"po{mt}_{hh}") for hh in range(2)]
            ot = uo_pool.tile([P, DV], BF, tag=f"ot{mt}")
            ot_tiles[mt] = ot

            def fin(h):
                lo, hi = H * h, H * (h + 1)
                # out = (psum * rowscale) * u fused on DVE
                nc.vector.scalar_tensor_tensor(
                    ot[:, lo:hi], po[h], rs_sb[:, mt:mt + 1], ut[:, lo:hi],
                    MUL, MUL)

            if tail_engines is None:
                for n in range(mt + 1):
                    for h in range(2):
                        nc.tensor.matmul(
                            po[h],
                            ats[n][:, P * j:P * (j + 1)],
                            v_tiles[n][:, H * h:H * (h + 1)],
                            start=(n == 0), stop=(n == mt),
                        )
                fin(0)
                fin(1)
            else:
                # tail variant: h-outer chains; fused DVE fin right after
                # each half-chain; each half stored immediately on its own
                # (drained) queue to shorten the tail
                for h in range(2):
                    lo, hi = H * h, H * (h + 1)
                    for n in range(mt + 1):
                        nc.tensor.matmul(
                            po[h],
                            ats[n][:, P * j:P * (j + 1)],
                            v_tiles[n][:, H * h:H * (h + 1)],
                            start=(n == 0), stop=(n == mt),
                        )
                    fin(h)
                    tail_engines[h].dma_start(
                        out=o_d[P * mt:P * (mt + 1), lo:hi],
                        in_=ot[:, lo:hi])

        def store(eng, mt):
            eng.dma_start(out=o_d[P * mt:P * (mt + 1), :], in_=ot_tiles[mt])

        # ================= emission =================
        # Load streams (per-engine order is what matters). Measured issue
        # holds: SP ~0.7-1.2us/DMA, Pool(SWDGE) ~2.7us, ACT ~0.7us (small
        # consts only). qT/kT load in halves so stage1 group 0/1 can start
        # after the first pair of half-loads.
        sp, pl = nc.sync, nc.gpsimd
        warm = cpool.tile([P, 2], BF, name="warm")
        nc.gpsimd.memset(warm, 0.0)
        sp.dma_start(out=kT[:, 0:L // 2], in_=kt_d[:, 0:L // 2])
        pl.dma_start(out=qT[:, 0:L // 2], in_=qt_d[:, 0:L // 2])
        nc.scalar.dma_start(out=kT[:, L // 2:L], in_=kt_d[:, L // 2:L])
        sp.dma_start(out=qT[:, L // 2:L], in_=qt_d[:, L // 2:L])
        nc.scalar.dma_start(out=tri, in_=tri_d)
        nc.scalar.dma_start(out=rs_sb, in_=rs_d)
        load_v(sp, 0)
        load_v(sp, 1)
        load_v(sp, 2)
        load_v(sp, 3)
        load_v(sp, 4)
        load_u(sp, 2)
        load_v(sp, 5)
        load_u(sp, 3)
        load_u(sp, 0)
        load_v(sp, 6)
        load_u(sp, 4)
        load_v(sp, 7)
        load_u(sp, 7)
        load_u(sp, 6)
        for n in range(8, 12):
            load_v(sp, n)
        load_u(sp, 8)
        for n in range(12, NT):
            load_v(sp, n)
        for mt in [11, 10, 9, 12, 15, 14, 13, 5, 1]:
            load_u(pl, mt)

        # ---- compute: a few tiny warmup matmuls start the PE p-state
        # ramp clock early; then stage1 interleaved between stage2 chains
        # so PE always has matmul work while ACT produces at-tiles; each
        # group's first-consumer chain is scheduled after its at
        # production ----
        ps_w = ps_o.tile([P, H], F32, tag="ps_o", name="ps_warm")
        for _ in range(3):
            nc.tensor.matmul(ps_w[0:2, 0:1], warm, warm[:, 0:1],
                             start=True, stop=True)
        stage1(0)
        stage1(1)
        stage2(2)
        stage1(2, range(0, 4))
        stage2(3)
        stage1(2, range(4, 8))
        stage2(0)
        stage1(2, range(8, 12))
        stage2(4)
        stage1(3, range(0, 4))
        stage2(7)
        stage1(3, range(4, 8))
        stage2(6)
        stage1(3, range(8, 12))
        stage2(8)
        stage1(3, range(12, NT))
        store(nc.scalar, 2)
        stage2(11)
        store(nc.scalar, 3)
        stage2(10)
        store(nc.scalar, 0)
        store(nc.scalar, 4)
        stage2(9)
        store(nc.scalar, 7)
        store(nc.scalar, 6)
        stage2(12)
        store(nc.scalar, 8)
        store(nc.scalar, 11)
        stage2(15)
        store(nc.scalar, 10)
        store(pl, 12)
        stage2(14)
        store(nc.scalar, 9)
        store(pl, 15)
        stage2(13)
        store(pl, 14)
        stage2(5, tail_engines=(pl, pl))
        store(nc.scalar, 13)
        stage2(1, tail_engines=(sp, sp))


def build_program():
    nc = bacc.Bacc("TRN2", target_bir_lowering=False, debug=False,
                   num_devices=B)
    io = declare_io(nc)
    with tile.TileContext(nc) as tc:
        build_kernel(nc, tc, io)
    nc.compile()
    return nc


_NC_CACHE = None


def kernel(u, q, k, v, attn_mask=None, trace=False):
    """Full inputs in, full output out. attn_mask ignored (deterministic
    causal)."""
    global _NC_CACHE
    if _NC_CACHE is None:
        _NC_CACHE = build_program()
    nc = _NC_CACHE

    in_maps = host_in_maps(q, k, v, u)
    res = run_bass_kernel_spmd(nc, in_maps, list(range(B)), trace=trace)
    out = np.stack([np.asarray(res.results[b]["out"]).astype(np.float32)
                    for b in range(B)])
    if trace:
        kernel.last_results = res
    return out
